# revision 1
# baseline (speedup 1.0000x reference)
"""Clustered Linformer Attention — TRN2 Bass kernel, batch-parallel over 8 NeuronCores.

Per core (one batch element b):
  A:  q^T = (x_b @ wq)^T ; k = x_b @ wk ; v = x_b @ wv        (PE, fp32r)
  B:  kp^T_h = k_h^T E_h ; vp^T_h = v_h^T F_h  (accumulated in PSUM across n-groups)
  C:  scores^T_h = kp_h q_h^T / 8  -> exp (ACT, fused scale)   [r x n strips]
  F:  out_raw^T_h = vp_h^T-contraction over r ; s_h = row-sums via ones-matmul
  N:  normalize via reciprocal + PE broadcast + DVE multiply
  G:  y = concat @ w_dense + b_dense (bias via K=1 ones matmul into PSUM)

All matmuls in float32r (full PE rate, ~1.5e-4 rel err). Stationary operands are
rounded by DVE/ACT copies (required by HW); moving operands only need the dtype.
"""
import sys
import numpy as np

for _p in ("/opt/trn_rl_repo", "/root/.axon_site/_ro/trn_rl_repo"):
    if _p not in sys.path:
        sys.path.insert(0, _p)

import concourse.bacc as bacc
import concourse.tile as tile
from concourse import mybir
from concourse.bass_utils import run_bass_kernel_spmd

B, N, D = 8, 4096, 512
H, R = 8, 256
DEP = D // H          # 64
P = 128
NG = 8                # n-groups for phase A/B
GN = N // NG          # 512 rows per group
NS = 8                # n-strips for phase C..G
SN = N // NS          # 512 cols per strip
F32 = mybir.dt.float32
F32R = mybir.dt.float32r
EXPF = mybir.ActivationFunctionType.Exp

_cache = {}


def build_program(repeat=1):
    key = ("nc", repeat)
    if key in _cache:
        return _cache[key]
    nc = bacc.Bacc("TRN2", target_bir_lowering=False, debug=False)
    x = nc.dram_tensor("x", [N, D], F32, kind="ExternalInput").ap()
    wq = nc.dram_tensor("wq", [D, D], F32, kind="ExternalInput").ap()
    wk = nc.dram_tensor("wk", [D, D], F32R, kind="ExternalInput").ap()
    wv = nc.dram_tensor("wv", [D, D], F32R, kind="ExternalInput").ap()
    wd = nc.dram_tensor("wd", [D, D], F32R, kind="ExternalInput").ap()
    bd = nc.dram_tensor("bd", [1, D], F32R, kind="ExternalInput").ap()
    E = nc.dram_tensor("E", [H, N, R], F32R, kind="ExternalInput").ap()
    Fm = nc.dram_tensor("F", [H, N, R], F32R, kind="ExternalInput").ap()
    ident_in = nc.dram_tensor("ident", [P, P], F32, kind="ExternalInput").ap()
    hb_in = nc.dram_tensor("hb", [P, D], F32, kind="ExternalInput").ap()
    ones_in = nc.dram_tensor("ones", [P, 1], F32, kind="ExternalInput").ap()
    bbc_in = nc.dram_tensor("b_bc", [P, D], F32R, kind="ExternalInput").ap()
    y = nc.dram_tensor("y", [N, D], F32, kind="ExternalOutput").ap()

    with tile.TileContext(nc) as tc, nc.allow_low_precision(reason="fp32r kernel"):
      for _rep in range(repeat):
        with tc.tile_pool(name="outer", bufs=1) as po:
            # ---- persistent tiles ----
            qT = [po.tile([P, N], F32R, tag=f"qT{c}", name=f"qT{c}") for c in range(4)]
            kpTz = [po.tile([P, R], F32R, tag=f"kpTz{h}", name=f"kpTz{h}") for h in range(H)]
            kpA = [po.tile([P, R], F32, tag=f"kpA{p}", name=f"kpA{p}") for p in range(4)]
            vpA = [po.tile([P, R], F32, tag=f"vpA{p}", name=f"vpA{p}") for p in range(4)]
            vp2 = [[po.tile([P, P], F32R, tag=f"vp2_{h}_{rc}", name=f"vp2_{h}_{rc}") for rc in range(2)]
                   for h in range(H)]
            zeros_t = po.tile([P, R], F32, tag="zeros", name="zeros")
            b_bc = po.tile([P, D], F32R, tag="b_bc", name="b_bc")
            wd_t = [po.tile([P, D], F32R, tag=f"wd{c}", name=f"wd{c}") for c in range(4)]
            ident = po.tile([P, P], F32, tag="ident", name="ident")
            hbr = [po.tile([P, P], F32R, tag=f"hb{p}", name=f"hb{p}") for p in range(4)]

            nc.sync.dma_start(ident[:], ident_in)
            nc.sync.dma_start(b_bc[:], bbc_in)
            for c in range(4):
                nc.sync.dma_start(wd_t[c][:], wd[c * P:(c + 1) * P, :])

            # ================= PHASE A+B =================
            with tc.tile_pool(name="pw", bufs=1) as pw, \
                 tc.tile_pool(name="pa", bufs=6) as pa, \
                 tc.tile_pool(name="pkv", bufs=6) as pkv, \
                 tc.tile_pool(name="pef", bufs=5) as pef, \
                 tc.tile_pool(name="psT", bufs=2, space="PSUM") as psT, \
                 tc.tile_pool(name="psB", bufs=1, space="PSUM") as psB:

                # constants that need rounding
                stage = pw.tile([P, D], F32, tag="hbstage", name="hbstage")
                nc.sync.dma_start(stage[:], hb_in)
                for p in range(4):
                    nc.vector.tensor_copy(hbr[p][:], stage[:, p * P:(p + 1) * P])
                ones_f = pw.tile([P, 1], F32, tag="onesstage", name="onesstage")
                nc.sync.dma_start(ones_f[:], ones_in)
                nc.gpsimd.memset(zeros_t[:], 0.0)

                # wq: fp32 load -> DVE round to fp32r (stationary)
                wqr = [pw.tile([P, D], F32R, tag=f"wqr{c}", name=f"wqr{c}") for c in range(4)]
                for c in range(4):
                    wq_raw = pw.tile([P, D], F32, tag="wqraw", name="wqraw")
                    nc.sync.dma_start(wq_raw[:], wq[c * P:(c + 1) * P, :])
                    nc.vector.tensor_copy(wqr[c][:], wq_raw[:])
                # wk / wv: straight fp32r loads (moving operands only)
                wk_t = [pw.tile([P, D], F32R, tag=f"wk{c}", name=f"wk{c}") for c in range(4)]
                wv_t = [pw.tile([P, D], F32R, tag=f"wv{c}", name=f"wv{c}") for c in range(4)]
                for c in range(4):
                    nc.sync.dma_start(wk_t[c][:], wk[c * P:(c + 1) * P, :])
                    nc.sync.dma_start(wv_t[c][:], wv[c * P:(c + 1) * P, :])

                for g in range(NG):
                    n0 = g * GN
                    xg_t = []
                    for i in range(4):
                        t = pa.tile([P, D], F32, tag="xg", name="xg")
                        nc.sync.dma_start(t[:], x[n0 + i * P:n0 + (i + 1) * P, :])
                        xg_t.append(t)
                    xT_t = [pa.tile([P, GN], F32R, tag="xT", name="xT") for c in range(4)]
                    for c in range(4):
                        for i in range(4):
                            tp = psT.tile([P, P], F32, tag="tp", name="tp")
                            nc.tensor.transpose(
                                tp[:], xg_t[i][:, c * P:(c + 1) * P], ident[:])
                            nc.scalar.copy(
                                xT_t[c][:, i * P:(i + 1) * P], tp[:])
                    # q^T
                    for dq in range(4):
                        qp = psT.tile([P, GN], F32, tag="qkv", name="qkv")
                        for c in range(4):
                            nc.tensor.matmul(
                                qp[:], wqr[c][:, dq * P:(dq + 1) * P], xT_t[c][:],
                                start=(c == 0), stop=(c == 3))
                        nc.scalar.copy(qT[dq][:, n0:n0 + GN], qp[:])
                    # k, v
                    kg_t = [pkv.tile([P, D], F32R, tag="kg", name="kg") for i in range(4)]
                    vg_t = [pkv.tile([P, D], F32R, tag="vg", name="vg") for i in range(4)]
                    for i in range(4):
                        kp_ = psT.tile([P, D], F32, tag="qkv", name="qkv")
                        for c in range(4):
                            nc.tensor.matmul(
                                kp_[:], xT_t[c][:, i * P:(i + 1) * P], wk_t[c][:],
                                start=(c == 0), stop=(c == 3))
                        nc.scalar.copy(kg_t[i][:], kp_[:])
                        vp_ = psT.tile([P, D], F32, tag="qkv", name="qkv")
                        for c in range(4):
                            nc.tensor.matmul(
                                vp_[:], xT_t[c][:, i * P:(i + 1) * P], wv_t[c][:],
                                start=(c == 0), stop=(c == 3))
                        nc.vector.tensor_copy(vg_t[i][:], vp_[:])
                    # B: project k, v through E_h, F_h.
                    # Full [128,128] stationary (both heads of the pair); the
                    # off-parity half of each matmul output is garbage and is
                    # simply never read by the DVE accumulate below.
                    for pidx in range(4):
                        kpg = [psB.tile([P, R], F32, tag=f"kpg{par}", name=f"kpg{par}")
                               for par in range(2)]
                        vpg = [psB.tile([P, R], F32, tag=f"vpg{par}", name=f"vpg{par}")
                               for par in range(2)]
                        for par in range(2):
                            h = 2 * pidx + par
                            Eh = pef.tile([P, 4, R], F32R, tag="ef", name="ef")
                            nc.sync.dma_start(
                                Eh[:], E[h, n0:n0 + GN, :].rearrange(
                                    "(i p) r -> p i r", p=P))
                            Fh = pef.tile([P, 4, R], F32R, tag="ef", name="ef")
                            nc.sync.dma_start(
                                Fh[:], Fm[h, n0:n0 + GN, :].rearrange(
                                    "(i p) r -> p i r", p=P))
                            for i in range(4):
                                nc.tensor.matmul(
                                    kpg[par][:],
                                    kg_t[i][:, pidx * P:(pidx + 1) * P],
                                    Eh[:, i, :],
                                    start=(i == 0), stop=(i == 3))
                            for i in range(4):
                                nc.tensor.matmul(
                                    vpg[par][:],
                                    vg_t[i][:, pidx * P:(pidx + 1) * P],
                                    Fh[:, i, :],
                                    start=(i == 0), stop=(i == 3))
                        for par in range(2):
                            ro = DEP * par
                            sl = slice(ro, ro + DEP)
                            if g == 0:
                                nc.vector.tensor_copy(kpA[pidx][sl, :], kpg[par][sl, :])
                                nc.vector.tensor_copy(vpA[pidx][sl, :], vpg[par][sl, :])
                            else:
                                nc.vector.tensor_add(
                                    kpA[pidx][sl, :], kpA[pidx][sl, :], kpg[par][sl, :])
                                nc.vector.tensor_add(
                                    vpA[pidx][sl, :], vpA[pidx][sl, :], vpg[par][sl, :])

                # evict kp into zero-padded per-head stationary tiles;
                # transpose vp pairs to natural layout (both heads per transpose)
                for p in range(4):
                    for par in range(2):
                        h = 2 * p + par
                        ro = DEP * par
                        oro = DEP * (1 - par)
                        nc.vector.tensor_copy(
                            kpTz[h][ro:ro + DEP, :], kpA[p][ro:ro + DEP, :])
                        nc.vector.tensor_copy(
                            kpTz[h][oro:oro + DEP, :], zeros_t[oro:oro + DEP, :])
                    for rc in range(2):
                        vt = psT.tile([P, P], F32, tag="tp", name="tp")
                        nc.tensor.transpose(
                            vt[:], vpA[p][:, rc * P:(rc + 1) * P], ident[:])
                        # vt[:, 0:64] = even head vp_nat, vt[:, 64:128] = odd
                        for par in range(2):
                            h = 2 * p + par
                            ro = DEP * par
                            oro = DEP * (1 - par)
                            # data columns at M-positions ro..ro+64
                            nc.vector.tensor_copy(
                                vp2[h][rc][:, ro:ro + DEP], vt[:, ro:ro + DEP])
                            # ones column at M-position oro (row-sum extractor);
                            # remaining columns zero
                            nc.vector.tensor_copy(
                                vp2[h][rc][:, oro:oro + 1], ones_f[:])
                            nc.vector.tensor_copy(
                                vp2[h][rc][:, oro + 1:oro + DEP],
                                zeros_t[:, 0:DEP - 1])

            # ================= PHASE C..G =================
            with tc.tile_pool(name="pexp", bufs=6) as pexp, \
                 tc.tile_pool(name="pstag", bufs=10) as pstag, \
                 tc.tile_pool(name="pcs", bufs=2) as pcs, \
                 tc.tile_pool(name="pbc", bufs=8) as pbc, \
                 tc.tile_pool(name="psml", bufs=4) as psml, \
                 tc.tile_pool(name="ps2", bufs=2, space="PSUM") as ps2, \
                 tc.tile_pool(name="ps1", bufs=1, space="PSUM") as ps1:
                for s in range(NS):
                    c0 = s * SN
                    csR = pcs.tile([P, 4, SN], F32R, tag="csR", name="csR")
                    S_t = psml.tile([P, SN], F32, tag="S", name="S")
                    nc.gpsimd.memset(S_t[:], 1.0)
                    stags = []
                    for h in range(H):
                        c = h // 2
                        ro = DEP * (h % 2)
                        oro = DEP * (1 - h % 2)
                        expT_t = [pexp.tile([P, SN], F32R, tag="expT", name="expT")
                                  for rc in range(2)]
                        for rc in range(2):
                            scp = ps2.tile([P, SN], F32, tag="sc", name="sc")
                            nc.tensor.matmul(
                                scp[:],
                                kpTz[h][:, rc * P:(rc + 1) * P],
                                qT[c][:, c0:c0 + SN],
                                start=True, stop=True)
                            nc.scalar.activation(
                                expT_t[rc][:], scp[:], EXPF,
                                scale=float(1.0 / np.sqrt(np.float32(DEP))))
                        fop = ps2.tile([P, SN], F32, tag="fo", name="fo")
                        for rc in range(2):
                            nc.tensor.matmul(
                                fop[:], vp2[h][rc][:], expT_t[rc][:],
                                start=(rc == 0), stop=(rc == 1))
                        stag = pstag.tile([P, SN], F32, tag="stag", name="stag")
                        nc.vector.tensor_copy(stag[:], fop[:])
                        nc.sync.dma_start(S_t[h:h + 1, :], stag[oro:oro + 1, :])
                        stags.append(stag)
                    Sr_t = psml.tile([P, SN], F32R, tag="Sr", name="Sr")
                    nc.vector.reciprocal(Sr_t[:], S_t[:])
                    for p in range(4):
                        bcp = ps1.tile([P, SN], F32, tag="bc", name="bc")
                        nc.tensor.matmul(bcp[:], hbr[p][:], Sr_t[:],
                                         start=True, stop=True)
                        bcs = pbc.tile([P, SN], F32, tag="bcs", name="bcs")
                        nc.scalar.copy(bcs[:], bcp[:])
                        for par in range(2):
                            h = 2 * p + par
                            ro = DEP * par
                            nc.gpsimd.tensor_mul(
                                csR[ro:ro + DEP, p, :],
                                stags[h][ro:ro + DEP, :],
                                bcs[ro:ro + DEP, :])
                    for j in range(4):
                        yp = ps2.tile([P, D], F32, tag="y", name="y")
                        for c in range(4):
                            nc.tensor.matmul(
                                yp[:], csR[:, c, j * P:(j + 1) * P], wd_t[c][:],
                                start=(c == 0), stop=(c == 3))
                        ys = psml.tile([P, D], F32, tag="ysb", name="ysb")
                        nc.vector.tensor_add(ys[:], yp[:], b_bc[:].bitcast(F32))
                        nc.sync.dma_start(y[c0 + j * P:c0 + (j + 1) * P, :], ys[:])

    nc.compile()
    _cache[key] = nc
    return nc


def make_in_maps(x, wq, wk, wv, E, F, w_dense, b_dense):
    x = np.ascontiguousarray(np.asarray(x, dtype=np.float32))
    consts = {
        "wq": np.ascontiguousarray(np.asarray(wq, np.float32)),
        "wk": np.ascontiguousarray(np.asarray(wk, np.float32)),
        "wv": np.ascontiguousarray(np.asarray(wv, np.float32)),
        "wd": np.ascontiguousarray(np.asarray(w_dense, np.float32)),
        "bd": np.ascontiguousarray(np.asarray(b_dense, np.float32)).reshape(1, D),
        "E": np.ascontiguousarray(np.asarray(E, np.float32)),
        "F": np.ascontiguousarray(np.asarray(F, np.float32)),
        "ident": np.eye(P, dtype=np.float32),
        "hb": _make_hb(),
        "ones": np.ones((P, 1), dtype=np.float32),
        "b_bc": np.tile(np.asarray(b_dense, np.float32).reshape(1, D), (P, 1)),
    }
    return [{"x": x[b], **consts} for b in range(B)]


def _make_hb():
    hb = np.zeros((P, D), dtype=np.float32)
    for p in range(4):
        for m in range(P):
            hb[2 * p + m // DEP, p * P + m] = 1.0
    return hb


def kernel(x, wq, wk, wv, E, F, w_dense, b_dense):
    nc = build_program()
    in_maps = make_in_maps(x, wq, wk, wv, E, F, w_dense, b_dense)
    res = run_bass_kernel_spmd(nc, in_maps, list(range(B)))
    out = np.stack([res.results[b]["y"] for b in range(B)], axis=0)
    return out.astype(np.float32)

